# revision 1
# baseline (speedup 1.0000x reference)
"""Trainium2 8-core Bass kernel for the AnyAttention relation-gate module.

Strategy: shard the sequence axis (q) across 8 cores (256 rows each).
All LayerNorm weights are folded into the projection matrices host-side.
Per core: LN stats -> normalized x (bf16) -> PE-transpose -> projections
(transposed layout) -> per-(b,g) transposed scores -> exp (ScalarE, no max
subtraction; scores are O(1) by construction) -> per-head O = P @ [v|vw2|1]
matmuls giving attention outputs plus row sums l and weighted sums u for
free -> tiny 16-float AllReduce for the global relation-gate sum -> per-
partition weighted recombination -> gate matmul -> sigmoid -> output.
"""

from contextlib import ExitStack

import numpy as np
import ml_dtypes

BF16 = ml_dtypes.bfloat16
LAST_RESULT = None

NLOC = 256          # q rows per core
B = 2
N = 2048
D = 256
G = 8
C = 32
NCORES = 8
TOKK = B * N        # 4096 (b-major full tokens)
TOKQ = B * NLOC     # 512  (b-major local q tokens)
EPS = 1e-5
SCALE = float(C) ** -0.5
CREL = 1.0 / ((N - 1) * D)   # relation-gate normalizer


def _build(consts, repeat=1):
    """Build the Bass graph. consts: dict of host-computed scalar immediates."""
    import concourse.bacc as bacc
    import concourse.bass as bass
    import concourse.mybir as mybir
    import concourse.tile as tile

    f32 = mybir.dt.float32
    bf16 = mybir.dt.bfloat16
    AX = mybir.AxisListType.X
    OP = mybir.AluOpType
    ACT = mybir.ActivationFunctionType

    nc = bacc.Bacc(
        "TRN2", target_bir_lowering=False, debug=False, num_devices=NCORES
    )

    def din(name, shape, dt=f32):
        return nc.dram_tensor(name, list(shape), dt, kind="ExternalInput")

    q_in = din("q_sl", (TOKQ, D))
    k_in = din("k_in", (TOKK, D))
    v_in = din("v_in", (TOKK, D))
    wqT = din("wqT", (D, D), bf16)       # (d, e), ln_q_w folded
    wkT = din("wkT", (D, D), bf16)
    wvT = din("wvT", (D, D + 1), bf16)   # cols 0..255 proj, col 256 = w2e
    w1Tm = din("w1Tm", (D, D), bf16)     # (d, e): -CREL*mlp_w[e]*W1[e,d]
    biasq = din("biasq", (128, 2))       # (e%128, etile)
    biask = din("biask", (128, 2))
    svec_bc = din("svec_bc", (128, D))   # broadcast rows
    mlb_bc = din("mlb_bc", (128, D))
    bv_bc = din("bv_bc", (128, D))
    identb = din("identb", (128, 128), bf16)
    identf = din("identf", (128, 128))
    ones_col = din("ones_col", (128, 1))
    ones_row = din("ones_row", (1, 128))
    out_t = nc.dram_tensor("out", [TOKQ, D], f32, kind="ExternalOutput")

    b2s_eff = consts["b2s_eff"]          # bias_v@w2sum + b2sum

    MS = bass.MemorySpace

    with tile.TileContext(nc) as tc:
        with (
            tc.tile_pool(name="const", bufs=1) as cpool,
            tc.tile_pool(name="persist", bufs=1) as ppool,
            tc.tile_pool(name="work", bufs=1) as wpool,
            tc.tile_pool(name="stats", bufs=1) as stpool,
            tc.tile_pool(name="small", bufs=1) as spool,
            tc.tile_pool(name="fin", bufs=2) as fpool,
            tc.tile_pool(name="pt", bufs=3) as ptpool,
            tc.tile_pool(name="dram", bufs=1, space=MS.DRAM) as dpool,
        ):
            # ---- load small constants ----
            identb_sb = cpool.tile([128, 128], bf16, tag="identb")
            identf_sb = cpool.tile([128, 128], f32, tag="identf")
            wq_sb = cpool.tile([128, 2, D], bf16, tag="wq")
            wk_sb = cpool.tile([128, 2, D], bf16, tag="wk")
            wv_sb = cpool.tile([128, 2, D + 1], bf16, tag="wv")
            w1_sb = cpool.tile([128, 2, D], bf16, tag="w1")
            biasq_sb = cpool.tile([128, 2], f32, tag="biasq")
            biask_sb = cpool.tile([128, 2], f32, tag="biask")
            svec_sb = cpool.tile([128, D], f32, tag="svec")
            mlb_sb = cpool.tile([128, D], f32, tag="mlb")
            bv_sb = cpool.tile([128, D], f32, tag="bv")
            onec_sb = cpool.tile([128, 1], f32, tag="onec")
            oner_sb = cpool.tile([1, 128], f32, tag="oner")
            nc.sync.dma_start(identb_sb[:], identb[:])
            nc.sync.dma_start(identf_sb[:], identf[:])
            nc.sync.dma_start(wq_sb[:], wqT.ap().rearrange("(c p) e -> p c e", p=128))
            nc.sync.dma_start(wk_sb[:], wkT.ap().rearrange("(c p) e -> p c e", p=128))
            nc.sync.dma_start(wv_sb[:], wvT.ap().rearrange("(c p) e -> p c e", p=128))
            nc.sync.dma_start(w1_sb[:], w1Tm.ap().rearrange("(c p) e -> p c e", p=128))
            nc.sync.dma_start(biasq_sb[:], biasq[:])
            nc.sync.dma_start(biask_sb[:], biask[:])
            nc.sync.dma_start(svec_sb[:], svec_bc[:])
            nc.sync.dma_start(mlb_sb[:], mlb_bc[:])
            nc.sync.dma_start(bv_sb[:], bv_bc[:])
            nc.sync.dma_start(onec_sb[:], ones_col[:])
            nc.sync.dma_start(oner_sb[:], ones_row[:])

            # persistent activation tensors
            kpT = ppool.tile([128, 2, TOKK], bf16, tag="kpT")
            qpT = ppool.tile([128, 2, TOKQ], bf16, tag="qpT")
            vp = ppool.tile([128, TOKK // 128, D + 2], bf16, tag="vp")
            og = ppool.tile([128, 16, 2, D], bf16, tag="og")       # (bg, qh, d)
            l_st = ppool.tile([128, 2, 16], f32, tag="l_st")
            u_st = ppool.tile([128, 2, 16], f32, tag="u_st")
            G1 = ppool.tile([128, 2, 2, D], f32, tag="G1")         # (b, qh, d)
            G2 = ppool.tile([128, 2, 2, D], f32, tag="G2")

            def _pipeline():
                # ============ Phase A+B: LN + transpose + projections ============
                stackA = ExitStack()
                psA = stackA.enter_context(
                    tc.tile_pool(name="psA", bufs=2, space=MS.PSUM)
                )

                def ln_and_transpose(src_ap, ntiles):
                    """Load natural f32, LN per row, produce transposed bf16 (d, tok)."""
                    raw = wpool.tile([128, TOKK // 128, D], f32, tag="raw")
                    nc.sync.dma_start(
                        raw[:, 0:ntiles, :], src_ap.rearrange("(i p) d -> p i d", p=128)
                    )
                    stats6 = stpool.tile([128, TOKK // 128, 6], f32, tag="stats6")
                    mv = stpool.tile([128, TOKK // 128, 2], f32, tag="mv")
                    veps = stpool.tile([128, TOKK // 128], f32, tag="veps")
                    std = stpool.tile([128, TOKK // 128], f32, tag="std")
                    mean = stpool.tile([128, TOKK // 128], f32, tag="mean")
                    rstd = stpool.tile([128, TOKK // 128], f32, tag="rstd")
                    for i in range(ntiles):
                        nc.vector.bn_stats(stats6[:, i, :], raw[:, i, :])
                        nc.vector.bn_aggr(mv[:, i, :], stats6[:, i, :])
                    sl = slice(0, ntiles)
                    nc.vector.tensor_copy(mean[:, sl], mv[:, sl, 0])
                    nc.vector.tensor_scalar(
                        veps[:, sl], mv[:, sl, 1], EPS, None, op0=OP.add
                    )
                    # rstd = 1/sqrt(var+eps)
                    nc.scalar.activation(std[:, sl], veps[:, sl], ACT.Sqrt)
                    nc.vector.reciprocal(rstd[:, sl], std[:, sl])
                    xhat = wpool.tile([128, TOKK // 128, D], bf16, tag="xhat")
                    for i in range(ntiles):
                        nc.vector.tensor_scalar(
                            xhat[:, i, :],
                            raw[:, i, :],
                            mean[:, i : i + 1],
                            rstd[:, i : i + 1],
                            op0=OP.subtract,
                            op1=OP.mult,
                        )
                    xT = wpool.tile([128, 2, TOKK], bf16, tag="xT")
                    for i in range(ntiles):
                        for dc in range(2):
                            nc.sync.dma_start_transpose(
                                xT[:, dc, 128 * i : 128 * i + 128],
                                xhat[:, i, 128 * dc : 128 * dc + 128],
                            )
                    return xT

                # K
                xkT = ln_and_transpose(k_in.ap(), TOKK // 128)
                for et in range(2):
                    for t in range(TOKK // 512):
                        ps = psA.tile([128, 512], f32, tag="proj")
                        for dc in range(2):
                            nc.tensor.matmul(
                                ps[:],
                                wk_sb[:, dc, 128 * et : 128 * et + 128],
                                xkT[:, dc, 512 * t : 512 * t + 512],
                                start=(dc == 0),
                                stop=(dc == 1),
                            )
                        nc.vector.tensor_scalar(
                            kpT[:, et, 512 * t : 512 * t + 512],
                            ps[:],
                            biask_sb[:, et : et + 1],
                            None,
                            op0=OP.add,
                        )
                # V (uses xvT as stationary operand; natural-layout output)
                xvT = ln_and_transpose(v_in.ap(), TOKK // 128)
                for t in range(TOKK // 128):
                    ps = psA.tile([128, D + 1], f32, tag="projv")
                    for dc in range(2):
                        nc.tensor.matmul(
                            ps[:],
                            xvT[:, dc, 128 * t : 128 * t + 128],
                            wv_sb[:, dc, :],
                            start=(dc == 0),
                            stop=(dc == 1),
                        )
                    nc.vector.tensor_copy(vp[:, t, 0 : D + 1], ps[:])
                nc.vector.memset(vp[:, :, D + 1 : D + 2], 1.0)
                # Q
                xqT = ln_and_transpose(q_in.ap(), TOKQ // 128)
                for et in range(2):
                    ps = psA.tile([128, 512], f32, tag="proj")
                    for dc in range(2):
                        nc.tensor.matmul(
                            ps[:],
                            wq_sb[:, dc, 128 * et : 128 * et + 128],
                            xqT[:, dc, 0:TOKQ],
                            start=(dc == 0),
                            stop=(dc == 1),
                        )
                    nc.vector.tensor_scalar(
                        qpT[:, et, :],
                        ps[:],
                        biasq_sb[:, et : et + 1],
                        None,
                        op0=OP.add,
                    )

                # ============ Phase C: attention per (b, g) ============
                import os as _os
                _stage = _os.environ.get("KSTAGE", "full")
                if _stage == "AB":
                    dummy = spool.tile([128, D], f32, tag="dummy")
                    nc.vector.memset(dummy[:], 0.0)
                    for r in range(4):
                        nc.sync.dma_start(out_t[128 * r : 128 * r + 128, :], dummy[:])
                    stackA.close()
                    return
                stackA.close()
                stackC = ExitStack()
                psS = stackC.enter_context(
                    tc.tile_pool(name="psS", bufs=2, space=MS.PSUM)
                )
                psO = stackC.enter_context(
                    tc.tile_pool(name="psO", bufs=4, space=MS.PSUM)
                )
                for b in range(B):
                    for g in range(G):
                        bg = 8 * b + g
                        gp = 32 * (g % 4)
                        et = g // 4
                        PT = ptpool.tile([128, 16, NLOC], bf16, tag="PT")
                        for quarter in range(4):
                            ps_s = psS.tile([128, 4, NLOC], f32, tag="scores")
                            for kc4 in range(4):
                                kc = 4 * quarter + kc4
                                nc.tensor.matmul(
                                    ps_s[:, kc4, :],
                                    kpT[gp : gp + 32, et,
                                        2048 * b + 128 * kc : 2048 * b + 128 * kc + 128],
                                    qpT[gp : gp + 32, et, NLOC * b : NLOC * b + NLOC],
                                    tile_position=(gp, 0),
                                )
                            nc.scalar.activation(
                                PT[:, 4 * quarter : 4 * quarter + 4, :],
                                ps_s[:, :, :],
                                ACT.Exp,
                                scale=SCALE,
                            )
                        for qh in range(2):
                            ps_o = psO.tile([128, D + 2], f32, tag="O")
                            for kc in range(16):
                                nc.tensor.matmul(
                                    ps_o[:],
                                    PT[:, kc, 128 * qh : 128 * qh + 128],
                                    vp[:, 16 * b + kc, :],
                                    start=(kc == 0),
                                    stop=(kc == 15),
                                )
                            nc.vector.tensor_copy(og[:, bg, qh, :], ps_o[:, 0:D])
                            nc.vector.tensor_copy(
                                u_st[:, qh, bg : bg + 1], ps_o[:, D : D + 1]
                            )
                            nc.vector.tensor_copy(
                                l_st[:, qh, bg : bg + 1], ps_o[:, D + 1 : D + 2]
                            )

                # ============ Phase D: small stage + collective ============
                if _stage == "C":
                    dummy = spool.tile([128, D], f32, tag="dummy")
                    nc.vector.memset(dummy[:], 0.0)
                    for r in range(4):
                        nc.sync.dma_start(out_t[128 * r : 128 * r + 128, :], dummy[:])
                    stackC.close()
                    return
                stackC.close()
                stackD = ExitStack()
                psB = stackD.enter_context(
                    tc.tile_pool(name="psB", bufs=1, space=MS.PSUM)
                )
                recl = spool.tile([128, 2, 16], f32, tag="recl")
                tt = spool.tile([128, 2, 16], f32, tag="tt")
                od = spool.tile([128, 2, 16], f32, tag="od")
                w1s = spool.tile([128, 2, 16], f32, tag="w1s")
                w2s = spool.tile([128, 2, 16], f32, tag="w2s")
                s_st = spool.tile([128, 2, 2], f32, tag="s_st")
                tp_sb = spool.tile([16, 1], f32, tag="tp_sb")
                trow = spool.tile([1, 16], f32, tag="trow")

                nc.vector.reciprocal(recl[:], l_st[:])
                nc.vector.tensor_mul(tt[:], u_st[:], recl[:])
                ps_tp = psB.tile([16, 1], f32, tag="tpp")
                for qh in range(2):
                    nc.tensor.matmul(
                        ps_tp[:],
                        tt[:, qh, :],
                        onec_sb[:],
                        start=(qh == 0),
                        stop=(qh == 1),
                    )
                nc.vector.tensor_copy(tp_sb[:], ps_tp[:])
                ar_in = dpool.tile([16, 1], f32, tag="ar_in")
                ar_out = dpool.tile([16, 1], f32, tag="ar_out")
                nc.sync.dma_start(ar_in[:], tp_sb[:])
                nc.gpsimd.collective_compute(
                    "AllReduce",
                    OP.add,
                    ins=[ar_in.opt()],
                    outs=[ar_out.opt()],
                    replica_groups=[list(range(NCORES))],
                )
                nc.sync.dma_start(trow[:], ar_out[:].rearrange("a b -> b a"))
                ps_tbc = psB.tile([128, 16], f32, tag="tbc")
                nc.tensor.matmul(ps_tbc[:], oner_sb[:], trow[:])
                for qh in range(2):
                    nc.vector.tensor_sub(od[:, qh, :], ps_tbc[:], tt[:, qh, :])
                nc.vector.tensor_scalar(
                    od[:], od[:], (N - 1) * b2s_eff, None, op0=OP.add
                )
                nc.vector.tensor_mul(w1s[:], od[:], recl[:])
                nc.vector.tensor_scalar_mul(w2s[:], recl[:], 1.0 / G)
                for b in range(B):
                    for qh in range(2):
                        nc.vector.reduce_sum(
                            s_st[:, qh, b : b + 1], od[:, qh, 8 * b : 8 * b + 8], axis=AX
                        )

                # ============ Phase E: weighted recombination ============
                for b in range(B):
                    for qh in range(2):
                        for g in range(G):
                            bg = 8 * b + g
                            if g == 0:
                                nc.vector.tensor_scalar_mul(
                                    G1[:, b, qh, :], og[:, bg, qh, :],
                                    w1s[:, qh, bg : bg + 1],
                                )
                                nc.vector.tensor_scalar_mul(
                                    G2[:, b, qh, :], og[:, bg, qh, :],
                                    w2s[:, qh, bg : bg + 1],
                                )
                            else:
                                nc.vector.affine_then_add(
                                    G1[:, b, qh, :], og[:, bg, qh, :], G1[:, b, qh, :],
                                    scale=w1s[:, qh, bg : bg + 1], bias=0.0,
                                )
                                nc.vector.affine_then_add(
                                    G2[:, b, qh, :], og[:, bg, qh, :], G2[:, b, qh, :],
                                    scale=w2s[:, qh, bg : bg + 1], bias=0.0,
                                )

                # ============ Phase F: gate + output ============
                g1T = ppool.tile([128, 2, 4, 128], bf16, tag="g1T")
                for b in range(B):
                    for qh in range(2):
                        idx = 2 * b + qh
                        for dc in range(2):
                            ps = psB.tile([128, 128], f32, tag="g1tp")
                            nc.tensor.matmul(
                                ps[:],
                                G1[:, b, qh, 128 * dc : 128 * dc + 128],
                                identf_sb[:],
                                is_transpose=True,
                            )
                            nc.vector.tensor_copy(g1T[:, dc, idx, :], ps[:])
                for b in range(B):
                    for qh in range(2):
                        idx = 2 * b + qh
                        ps_a = psB.tile([128, D], f32, tag="A0")
                        for dc in range(2):
                            nc.tensor.matmul(
                                ps_a[:],
                                g1T[:, dc, idx, :],
                                w1_sb[:, dc, :],
                                start=(dc == 0),
                                stop=(dc == 1),
                            )
                        gi = fpool.tile([128, D], f32, tag="gi")
                        nc.vector.affine_then_add(
                            gi[:], svec_sb[:], ps_a[:],
                            scale=s_st[:, qh, b : b + 1], bias=0.0,
                        )
                        nc.vector.tensor_add(gi[:], gi[:], mlb_sb[:])
                        gate = fpool.tile([128, D], f32, tag="gate")
                        nc.scalar.activation(gate[:], gi[:], ACT.Sigmoid)
                        of = fpool.tile([128, D], f32, tag="of")
                        nc.vector.tensor_add(of[:], G2[:, b, qh, :], bv_sb[:])
                        nc.vector.tensor_mul(of[:], of[:], gate[:])
                        nc.sync.dma_start(
                            out_t[NLOC * b + 128 * qh : NLOC * b + 128 * qh + 128, :],
                            of[:],
                        )
                stackD.close()

            for _rep in range(repeat):
                _pipeline()

    return nc


def prepare(**inputs):
    q = np.asarray(inputs["q"], np.float32)
    k = np.asarray(inputs["k"], np.float32)
    v = np.asarray(inputs["v"], np.float32)
    ln_q_w = np.asarray(inputs["ln_q_w"], np.float64)
    ln_q_b = np.asarray(inputs["ln_q_b"], np.float64)
    ln_k_w = np.asarray(inputs["ln_k_w"], np.float64)
    ln_k_b = np.asarray(inputs["ln_k_b"], np.float64)
    ln_v_w = np.asarray(inputs["ln_v_w"], np.float64)
    ln_v_b = np.asarray(inputs["ln_v_b"], np.float64)
    Wq = np.asarray(inputs["Wq"], np.float64)
    Wk = np.asarray(inputs["Wk"], np.float64)
    Wv = np.asarray(inputs["Wv"], np.float64)
    W1 = np.asarray(inputs["W1"], np.float64)
    b1 = np.asarray(inputs["b1"], np.float64)
    W2 = np.asarray(inputs["W2"], np.float64)
    b2 = np.asarray(inputs["b2"], np.float64)
    mlp_w = np.asarray(inputs["mlp_w"], np.float64)
    mlp_b = np.asarray(inputs["mlp_b"], np.float64)

    # ---- host-side weight folding ----
    wqT = (Wq.T * ln_q_w[:, None]).astype(BF16)            # (d, e)
    wkT = (Wk.T * ln_k_w[:, None]).astype(BF16)
    wvT_eff = Wv.T * ln_v_w[:, None]                        # (d, e) float64
    biasq = (ln_q_b @ Wq.T).astype(np.float32)              # (e,)
    biask = (ln_k_b @ Wk.T).astype(np.float32)
    bias_v = ln_v_b @ Wv.T                                  # (e,) float64
    w2sum = W2.sum(axis=0)                                  # (d,)
    b2sum = float(b2.sum())
    w2e = wvT_eff @ w2sum                                   # (d,)
    wvT = np.concatenate([wvT_eff, w2e[:, None]], axis=1).astype(BF16)  # (d, 257)
    w1Tm_f = W1.T * (-CREL * mlp_w)[None, :]                # (d, e) float64
    w1Tm = w1Tm_f.astype(BF16)
    svec = (-CREL * mlp_w * b1 + bias_v @ w1Tm_f).astype(np.float32)
    b2s_eff = float(bias_v @ w2sum + b2sum)

    def bc(vec):
        return np.broadcast_to(
            np.asarray(vec, np.float32)[None, :], (128, D)
        ).copy()

    biasq_d = np.ascontiguousarray(biasq.reshape(2, 128).T)  # (128, 2)
    biask_d = np.ascontiguousarray(biask.reshape(2, 128).T)

    # ---- b-major activations ----
    k_bm = np.ascontiguousarray(k.transpose(1, 0, 2).reshape(TOKK, D))
    v_bm = np.ascontiguousarray(v.transpose(1, 0, 2).reshape(TOKK, D))
    q_bm = q.transpose(1, 0, 2)                             # (B, N, D)

    common = {
        "k_in": k_bm,
        "v_in": v_bm,
        "wqT": wqT,
        "wkT": wkT,
        "wvT": wvT,
        "w1Tm": w1Tm,
        "biasq": biasq_d,
        "biask": biask_d,
        "svec_bc": bc(svec),
        "mlb_bc": bc(mlp_b),
        "bv_bc": bc(bias_v),
        "identb": np.eye(128, dtype=BF16),
        "identf": np.eye(128, dtype=np.float32),
        "ones_col": np.ones((128, 1), np.float32),
        "ones_row": np.ones((1, 128), np.float32),
    }
    in_maps = []
    for i in range(NCORES):
        q_sl = np.ascontiguousarray(
            q_bm[:, i * NLOC : (i + 1) * NLOC, :].reshape(TOKQ, D)
        )
        in_maps.append({**common, "q_sl": q_sl})

    return in_maps, {"b2s_eff": b2s_eff}


def assemble(results):
    full = np.zeros((B, N, D), np.float32)
    for i in range(NCORES):
        o = np.asarray(results[i]["out"]).reshape(B, NLOC, D)
        full[:, i * NLOC : (i + 1) * NLOC, :] = o
    return np.ascontiguousarray(full.transpose(1, 0, 2))


def kernel(**inputs):
    from concourse import bass_utils

    in_maps, consts = prepare(**inputs)
    nc = _build(consts)
    nc.compile()
    res = bass_utils.run_bass_kernel_spmd(nc, in_maps, core_ids=list(range(NCORES)))
    global LAST_RESULT
    LAST_RESULT = res
    return assemble([res.results[i] for i in range(NCORES)])



# revision 13
# speedup vs baseline: 1.5759x; 1.5759x over previous
"""Trainium2 8-core Bass kernel for the AnyAttention relation-gate module.

Sequence-sharded (256 q rows/core). Key structural choices vs the naive
version:
- LayerNorm mean-term folded into column-centered projection weights, so
  only the 1/std per-token scale is applied on device (one tensor_scalar).
- Transposition of normalized activations done via two big DRAM-roundtrip
  xbar DMA transposes per tensor half instead of 136 small SBUF ones.
- Scores matmuls for head pairs interleave PE row-quadrants (K=32 packing).
- Per-head attention outputs are evicted from PSUM directly as normalized
  Y_g = O_g / l_g; the relation-gate recombination is expressed as
  G1 = sum_g Ttil_g*Y_g - Z with Z = sum_g tt_g*Y_g accumulated before the
  16-float AllReduce, leaving only the Ttil part on the post-collective tail.
"""

from contextlib import ExitStack

import numpy as np
import ml_dtypes

BF16 = ml_dtypes.bfloat16
LAST_RESULT = None

NLOC = 256          # q rows per core
B = 2
N = 2048
D = 256
G = 8
C = 32
NCORES = 8
TOKK = B * N        # 4096 (b-major full tokens)
TOKQ = B * NLOC     # 512  (b-major local q tokens)
EPS = 1e-5
SCALE = float(C) ** -0.5
CREL = 1.0 / ((N - 1) * D)   # relation-gate normalizer


def _build(consts):
    import concourse.bacc as bacc
    import concourse.bass as bass
    import concourse.mybir as mybir
    import concourse.tile as tile

    f32 = mybir.dt.float32
    bf16 = mybir.dt.bfloat16
    AX = mybir.AxisListType.X
    OP = mybir.AluOpType
    ACT = mybir.ActivationFunctionType

    nc = bacc.Bacc(
        "TRN2", target_bir_lowering=False, debug=False, num_devices=NCORES
    )

    def din(name, shape, dt=f32):
        return nc.dram_tensor(name, list(shape), dt, kind="ExternalInput")

    q_in = din("q_nb", (TOKQ, D), bf16)
    k_in = din("k_nb", (TOKK, D), bf16)
    v_in = din("v_nb", (TOKK, D), bf16)
    wqc = din("wqc", (D, D), bf16)        # (d, e), centered + ln_w folded
    wkc = din("wkc", (D, D), bf16)
    wvc = din("wvc", (D, D + 1), bf16)    # cols 0..255 proj, col 256 = w2e
    w1Tm = din("w1Tm", (D, D), bf16)      # (d, e): -CREL*mlp_w[e]*W1[e,d]
    biasq = din("biasq", (128, 2))        # (e%128, etile)
    biask = din("biask", (128, 2))
    svec_bc = din("svec_bc", (128, D))    # broadcast rows
    mlb_bc = din("mlb_bc", (128, D))
    bv_bc = din("bv_bc", (128, D))
    identf = din("identf", (128, 128))
    ones_col = din("ones_col", (128, 1))
    ones_row = din("ones_row", (1, 128))
    out_t = nc.dram_tensor("out", [TOKQ, D], f32, kind="ExternalOutput")

    b2s_eff = consts["b2s_eff"]

    MS = bass.MemorySpace

    with tile.TileContext(nc) as tc:
        with (
            tc.tile_pool(name="const", bufs=1) as cpool,
            tc.tile_pool(name="persist", bufs=1) as ppool,
            tc.tile_pool(name="work", bufs=2) as wpool,
            tc.tile_pool(name="xt", bufs=1) as xtpool,
            tc.tile_pool(name="stats", bufs=3) as stpool,
            tc.tile_pool(name="small", bufs=1) as spool,
            tc.tile_pool(name="fin", bufs=2) as fpool,
            tc.tile_pool(name="pt", bufs=2) as ptpool,
            tc.tile_pool(name="dram", bufs=1, space=MS.DRAM) as dpool,
        ):
            # ---- small constants ----
            identf_sb = cpool.tile([128, 128], f32, tag="identf")
            wq_sb = cpool.tile([128, 2, D], bf16, tag="wq")
            wk_sb = cpool.tile([128, 2, D], bf16, tag="wk")
            wv_sb = cpool.tile([128, 2, D + 1], bf16, tag="wv")
            w1_sb = cpool.tile([128, 2, D], bf16, tag="w1")
            biasq_sb = cpool.tile([128, 2], f32, tag="biasq")
            biask_sb = cpool.tile([128, 2], f32, tag="biask")
            svec_sb = cpool.tile([128, D], f32, tag="svec")
            mlb_sb = cpool.tile([128, D], f32, tag="mlb")
            bv_sb = cpool.tile([128, D], f32, tag="bv")
            onec_sb = cpool.tile([128, 1], f32, tag="onec")
            oner_sb = cpool.tile([1, 128], f32, tag="oner")
            nc.sync.dma_start(identf_sb[:], identf[:])
            nc.sync.dma_start(wq_sb[:], wqc.ap().rearrange("(c p) e -> p c e", p=128))
            nc.sync.dma_start(wk_sb[:], wkc.ap().rearrange("(c p) e -> p c e", p=128))
            nc.sync.dma_start(wv_sb[:], wvc.ap().rearrange("(c p) e -> p c e", p=128))
            nc.sync.dma_start(w1_sb[:], w1Tm.ap().rearrange("(c p) e -> p c e", p=128))
            nc.sync.dma_start(biasq_sb[:], biasq[:])
            nc.sync.dma_start(biask_sb[:], biask[:])
            nc.sync.dma_start(svec_sb[:], svec_bc[:])
            nc.sync.dma_start(mlb_sb[:], mlb_bc[:])
            nc.sync.dma_start(bv_sb[:], bv_bc[:])
            nc.sync.dma_start(onec_sb[:], ones_col[:])
            nc.sync.dma_start(oner_sb[:], ones_row[:])

            # DRAM scratch for the transpose roundtrips (dh-major)
            khat_d = dpool.tile([2, TOKK, 128], bf16, tag="khat_d")
            vhat_d = dpool.tile([2, TOKK, 128], bf16, tag="vhat_d")
            qhat_d = dpool.tile([2, TOKQ, 128], bf16, tag="qhat_d")

            # persistent activation tensors
            kpT = ppool.tile([128, 2, TOKK], bf16, tag="kpT")
            qpT = ppool.tile([128, 2, TOKQ], bf16, tag="qpT")
            vp = ppool.tile([128, TOKK // 128, D + 2], bf16, tag="vp")
            Yall = ppool.tile([128, 16, 2, D], bf16, tag="Yall")   # (bg, qh, d)
            recl = ppool.tile([128, 2, 16], f32, tag="recl")
            ttl = ppool.tile([128, 2, 16], f32, tag="ttl")
            Zt = ppool.tile([128, 2, 2, D], f32, tag="Zt")         # (b, qh, d)
            G1 = ppool.tile([128, 2, 2, D], f32, tag="G1")
            G2 = ppool.tile([128, 2, 2, D], f32, tag="G2")

            nc.vector.memset(Zt[:], 0.0)
            nc.vector.memset(G2[:], 0.0)
            nc.vector.memset(vp[:, :, D + 1 : D + 2], 1.0)

            stackA = ExitStack()
            psA = stackA.enter_context(
                tc.tile_pool(name="psA", bufs=2, space=MS.PSUM)
            )

            # ---------- phase A helpers ----------
            def stats_xhat_wb(src_ap, scratch_d, nt, b):
                """Per half-b: load raw bf16, bn stats, xhat = x * rstd,
                write xhat back to DRAM scratch split by d-half."""
                t0 = 16 * b
                raw = wpool.tile([128, TOKK // 256, D], bf16, tag="raw")
                nc.sync.dma_start(
                    raw[:, 0:nt, :],
                    src_ap.rearrange("(i p) d -> p i d", p=128)[:, t0 : t0 + nt, :],
                )
                stats6 = stpool.tile([128, TOKK // 256, 6], f32, tag="stats6")
                mv = stpool.tile([128, TOKK // 256, 2], f32, tag="mv")
                veps = stpool.tile([128, TOKK // 256], f32, tag="veps")
                std = stpool.tile([128, TOKK // 256], f32, tag="std")
                rr = stpool.tile([128, TOKK // 256], f32, tag="rr")
                for i in range(nt):
                    nc.vector.bn_stats(stats6[:, i, :], raw[:, i, :])
                    nc.vector.bn_aggr(mv[:, i, :], stats6[:, i, :])
                sl = slice(0, nt)
                nc.vector.tensor_scalar(
                    veps[:, sl], mv[:, sl, 1], EPS, None, op0=OP.add
                )
                nc.scalar.activation(std[:, sl], veps[:, sl], ACT.Sqrt)
                nc.vector.reciprocal(rr[:, sl], std[:, sl])
                xhat = wpool.tile([128, TOKK // 256, D], bf16, tag="xhat")
                for i in range(nt):
                    nc.vector.tensor_scalar(
                        xhat[:, i, :],
                        raw[:, i, :],
                        rr[:, i : i + 1],
                        None,
                        op0=OP.mult,
                    )
                for dh in range(2):
                    nc.sync.dma_start(
                        scratch_d[dh, 128 * t0 : 128 * (t0 + nt), :].rearrange(
                            "(i p) c -> p i c", p=128
                        ),
                        xhat[:, 0:nt, 128 * dh : 128 * dh + 128],
                    )

            def transpose_half(scratch_d, xT, b, ntok):
                for dh in range(2):
                    nc.sync.dma_start_transpose(
                        xT[:, dh, ntok * b : ntok * (b + 1)],
                        scratch_d[dh, ntok * b : ntok * (b + 1), :],
                    )

            # ---------- Q (small, do first) ----------
            stats_xhat_wb(q_in.ap(), qhat_d, 4, 0)
            xqT = xtpool.tile([128, 2, TOKQ], bf16, tag="xqT")
            transpose_half(qhat_d, xqT, 0, TOKQ)
            for et in range(2):
                ps = psA.tile([128, 512], f32, tag="proj")
                for dh in range(2):
                    nc.tensor.matmul(
                        ps[:],
                        wq_sb[:, dh, 128 * et : 128 * et + 128],
                        xqT[:, dh, :],
                        start=(dh == 0),
                        stop=(dh == 1),
                    )
                nc.scalar.activation(
                    qpT[:, et, :], ps[:], ACT.Identity, bias=biasq_sb[:, et : et + 1]
                )

            # ---------- K, V per b-half ----------
            xkT = xtpool.tile([128, 2, TOKK], bf16, tag="xkT")
            xvT = xtpool.tile([128, 2, TOKK], bf16, tag="xvT")

            def k_half(b):
                stats_xhat_wb(k_in.ap(), khat_d, 16, b)
                transpose_half(khat_d, xkT, b, 2048)
                for et in range(2):
                    for t in range(4):
                        ps = psA.tile([128, 512], f32, tag="proj")
                        for dh in range(2):
                            nc.tensor.matmul(
                                ps[:],
                                wk_sb[:, dh, 128 * et : 128 * et + 128],
                                xkT[:, dh, 2048 * b + 512 * t : 2048 * b + 512 * t + 512],
                                start=(dh == 0),
                                stop=(dh == 1),
                            )
                        nc.scalar.activation(
                            kpT[:, et, 2048 * b + 512 * t : 2048 * b + 512 * t + 512],
                            ps[:],
                            ACT.Identity,
                            bias=biask_sb[:, et : et + 1],
                        )

            def v_half(b):
                stats_xhat_wb(v_in.ap(), vhat_d, 16, b)
                transpose_half(vhat_d, xvT, b, 2048)
                for t in range(16):
                    tt_ = 16 * b + t
                    ps = psA.tile([128, D + 1], f32, tag="projv")
                    for dh in range(2):
                        nc.tensor.matmul(
                            ps[:],
                            xvT[:, dh, 128 * tt_ : 128 * tt_ + 128],
                            wv_sb[:, dh, :],
                            start=(dh == 0),
                            stop=(dh == 1),
                        )
                    nc.vector.tensor_copy(vp[:, tt_, 0 : D + 1], ps[:])

            k_half(0)
            v_half(0)
            k_half(1)
            v_half(1)

            # ---------- phase C: attention ----------
            import os as _os
            _stage = _os.environ.get("KSTAGE", "full")
            if _stage == "AB":
                dummy = spool.tile([128, D], f32, tag="dummy")
                nc.vector.memset(dummy[:], 0.0)
                for r in range(4):
                    nc.sync.dma_start(out_t[128 * r : 128 * r + 128, :], dummy[:])
                stackA.close()
                return nc
            stackA.close()

            stackC = ExitStack()
            psS = stackC.enter_context(
                tc.tile_pool(name="psS", bufs=3, space=MS.PSUM)
            )
            psO = stackC.enter_context(
                tc.tile_pool(name="psO", bufs=2, space=MS.PSUM)
            )
            for b in range(B):
                for pair in range(4):
                    g0, g1 = 2 * pair, 2 * pair + 1
                    PT0 = ptpool.tile([128, 16, NLOC], bf16, tag="pt0")
                    PT1 = ptpool.tile([128, 16, NLOC], bf16, tag="pt1")
                    for quarter in range(4):
                        ts0 = psS.tile([128, 4, NLOC], f32, tag="sc")
                        ts1 = psS.tile([128, 4, NLOC], f32, tag="sc")
                        for kc4 in range(4):
                            kc = 4 * quarter + kc4
                            for gg, tsx in ((g0, ts0), (g1, ts1)):
                                gp = 32 * (gg % 4)
                                et = gg // 4
                                nc.tensor.matmul(
                                    tsx[:, kc4, :],
                                    kpT[gp : gp + 32, et,
                                        2048 * b + 128 * kc : 2048 * b + 128 * kc + 128],
                                    qpT[gp : gp + 32, et, NLOC * b : NLOC * b + NLOC],
                                    tile_position=(gp, 0),
                                )
                        nc.scalar.activation(
                            PT0[:, 4 * quarter : 4 * quarter + 4, :],
                            ts0[:, :, :],
                            ACT.Exp,
                            scale=SCALE,
                        )
                        nc.scalar.activation(
                            PT1[:, 4 * quarter : 4 * quarter + 4, :],
                            ts1[:, :, :],
                            ACT.Exp,
                            scale=SCALE,
                        )
                    for gg, PT in ((g0, PT0), (g1, PT1)):
                        bg = 8 * b + gg
                        for qh in range(2):
                            ps_o = psO.tile([128, D + 2], f32, tag="O")
                            for kc in range(16):
                                nc.tensor.matmul(
                                    ps_o[:],
                                    PT[:, kc, 128 * qh : 128 * qh + 128],
                                    vp[:, 16 * b + kc, :],
                                    start=(kc == 0),
                                    stop=(kc == 15),
                                )
                            rsl = recl[:, qh, bg : bg + 1]
                            nc.vector.reciprocal(rsl, ps_o[:, D + 1 : D + 2])
                            nc.vector.tensor_mul(
                                ttl[:, qh, bg : bg + 1], ps_o[:, D : D + 1], rsl
                            )
                            nc.vector.tensor_scalar(
                                Yall[:, bg, qh, :],
                                ps_o[:, 0:D],
                                rsl,
                                None,
                                op0=OP.mult,
                            )
                            # Z += tt * Y (vector), G2 += Y (gpsimd)
                            nc.vector.scalar_tensor_tensor(
                                Zt[:, b, qh, :],
                                Yall[:, bg, qh, :],
                                ttl[:, qh, bg : bg + 1],
                                Zt[:, b, qh, :],
                                op0=OP.mult,
                                op1=OP.add,
                            )
                            nc.gpsimd.tensor_add(
                                G2[:, b, qh, :], Yall[:, bg, qh, :], G2[:, b, qh, :]
                            )

            if _stage == "C":
                dummy = spool.tile([128, D], f32, tag="dummy")
                nc.vector.memset(dummy[:], 0.0)
                for r in range(4):
                    nc.sync.dma_start(out_t[128 * r : 128 * r + 128, :], dummy[:])
                stackC.close()
                return nc
            stackC.close()

            # ---------- phase D: collective ----------
            stackD = ExitStack()
            psB = stackD.enter_context(
                tc.tile_pool(name="psB", bufs=2, space=MS.PSUM)
            )
            tp_sb = spool.tile([16, 1], f32, tag="tp_sb")
            trow = spool.tile([1, 16], f32, tag="trow")
            Ttil = spool.tile([128, 16], f32, tag="Ttil")
            s_st = spool.tile([128, 2, 2], f32, tag="s_st")
            sg_t = spool.tile([128, 2, 2], f32, tag="sg_t")
            sg_T = spool.tile([128, 2], f32, tag="sg_T")
            scr = spool.tile([128, 1], f32, tag="scr")

            ps_tp = psB.tile([16, 1], f32, tag="tpp")
            for qh in range(2):
                nc.tensor.matmul(
                    ps_tp[:],
                    ttl[:, qh, :],
                    onec_sb[:],
                    start=(qh == 0),
                    stop=(qh == 1),
                )
            nc.vector.tensor_copy(tp_sb[:], ps_tp[:])
            # hide the sigmoid ACT-table load inside the collective window
            # (dep on the LAST tt write so it can't land mid-phase-C)
            nc.scalar.activation(scr[:], ttl[:, 1, 15:16], ACT.Sigmoid)
            ar_in = dpool.tile([16, 1], f32, tag="ar_in")
            ar_out = dpool.tile([16, 1], f32, tag="ar_out")
            nc.sync.dma_start(ar_in[:], tp_sb[:])
            nc.gpsimd.collective_compute(
                "AllReduce",
                OP.add,
                ins=[ar_in.opt()],
                outs=[ar_out.opt()],
                replica_groups=[list(range(NCORES))],
            )
            nc.sync.dma_start(trow[:], ar_out[:].rearrange("a b -> b a"))
            ps_tbc = psB.tile([128, 16], f32, tag="tbc")
            nc.tensor.matmul(ps_tbc[:], oner_sb[:], trow[:])
            nc.vector.tensor_scalar(
                Ttil[:], ps_tbc[:], (N - 1) * b2s_eff, None, op0=OP.add
            )
            # s_st = sum_g Ttil - sum_g tt   (per b, qh)
            for b in range(B):
                nc.vector.reduce_sum(
                    sg_T[:, b : b + 1], Ttil[:, 8 * b : 8 * b + 8], axis=AX
                )
                for qh in range(2):
                    nc.vector.reduce_sum(
                        sg_t[:, qh, b : b + 1], ttl[:, qh, 8 * b : 8 * b + 8], axis=AX
                    )
                    nc.vector.tensor_sub(
                        s_st[:, qh, b : b + 1], sg_T[:, b : b + 1], sg_t[:, qh, b : b + 1]
                    )

            # ---------- phase E: G1 = sum_g Ttil_g * Y_g - Z ----------
            for b in range(B):
                for qh in range(2):
                    eng = nc.vector
                    nc.vector.tensor_scalar_mul(G1[:, b, qh, :], Zt[:, b, qh, :], -1.0)
                    for g in range(G):
                        bg = 8 * b + g
                        eng.scalar_tensor_tensor(
                            G1[:, b, qh, :],
                            Yall[:, bg, qh, :],
                            Ttil[:, bg : bg + 1],
                            G1[:, b, qh, :],
                            op0=OP.mult,
                            op1=OP.add,
                        )

            # ---------- phase F: gate + output ----------
            g1T = ppool.tile([128, 2, 4, 128], bf16, tag="g1T")
            for b in range(B):
                for qh in range(2):
                    idx = 2 * b + qh
                    for dh in range(2):
                        ps = psB.tile([128, 128], f32, tag="g1tp")
                        nc.tensor.matmul(
                            ps[:],
                            G1[:, b, qh, 128 * dh : 128 * dh + 128],
                            identf_sb[:],
                            is_transpose=True,
                        )
                        nc.vector.tensor_copy(g1T[:, dh, idx, :], ps[:])
            for b in range(B):
                for qh in range(2):
                    idx = 2 * b + qh
                    ps_a = psB.tile([128, D], f32, tag="A0")
                    for dh in range(2):
                        nc.tensor.matmul(
                            ps_a[:],
                            g1T[:, dh, idx, :],
                            w1_sb[:, dh, :],
                            start=(dh == 0),
                            stop=(dh == 1),
                        )
                    gi = fpool.tile([128, D], f32, tag="gi")
                    nc.vector.scalar_tensor_tensor(
                        gi[:],
                        svec_sb[:],
                        s_st[:, qh, b : b + 1],
                        ps_a[:],
                        op0=OP.mult,
                        op1=OP.add,
                    )
                    nc.vector.tensor_add(gi[:], gi[:], mlb_sb[:])
                    gate = fpool.tile([128, D], f32, tag="gate")
                    nc.scalar.activation(gate[:], gi[:], ACT.Sigmoid)
                    of = fpool.tile([128, D], f32, tag="of")
                    nc.vector.scalar_tensor_tensor(
                        of[:],
                        G2[:, b, qh, :],
                        1.0 / G,
                        bv_sb[:],
                        op0=OP.mult,
                        op1=OP.add,
                    )
                    nc.vector.tensor_mul(of[:], of[:], gate[:])
                    nc.sync.dma_start(
                        out_t[NLOC * b + 128 * qh : NLOC * b + 128 * qh + 128, :],
                        of[:],
                    )
            stackD.close()

    return nc


def prepare(**inputs):
    q = np.asarray(inputs["q"], np.float32)
    k = np.asarray(inputs["k"], np.float32)
    v = np.asarray(inputs["v"], np.float32)
    ln_q_w = np.asarray(inputs["ln_q_w"], np.float64)
    ln_q_b = np.asarray(inputs["ln_q_b"], np.float64)
    ln_k_w = np.asarray(inputs["ln_k_w"], np.float64)
    ln_k_b = np.asarray(inputs["ln_k_b"], np.float64)
    ln_v_w = np.asarray(inputs["ln_v_w"], np.float64)
    ln_v_b = np.asarray(inputs["ln_v_b"], np.float64)
    Wq = np.asarray(inputs["Wq"], np.float64)
    Wk = np.asarray(inputs["Wk"], np.float64)
    Wv = np.asarray(inputs["Wv"], np.float64)
    W1 = np.asarray(inputs["W1"], np.float64)
    b1 = np.asarray(inputs["b1"], np.float64)
    W2 = np.asarray(inputs["W2"], np.float64)
    b2 = np.asarray(inputs["b2"], np.float64)
    mlp_w = np.asarray(inputs["mlp_w"], np.float64)
    mlp_b = np.asarray(inputs["mlp_b"], np.float64)

    # ---- host-side weight folding ----
    wqT = Wq.T * ln_q_w[:, None]            # (d, e)
    wkT = Wk.T * ln_k_w[:, None]
    wvT = Wv.T * ln_v_w[:, None]
    biasq = (ln_q_b @ Wq.T).astype(np.float32)
    biask = (ln_k_b @ Wk.T).astype(np.float32)
    bias_v = ln_v_b @ Wv.T
    w2sum = W2.sum(axis=0)
    b2sum = float(b2.sum())
    w2e = wvT @ w2sum
    wv_ext = np.concatenate([wvT, w2e[:, None]], axis=1)      # (d, 257)
    # column-centered weights: projection of raw x equals projection of
    # mean-centered x
    wqc = (wqT - wqT.mean(0, keepdims=True)).astype(BF16)
    wkc = (wkT - wkT.mean(0, keepdims=True)).astype(BF16)
    wvc = (wv_ext - wv_ext.mean(0, keepdims=True)).astype(BF16)
    w1Tm_f = W1.T * (-CREL * mlp_w)[None, :]
    w1Tm = w1Tm_f.astype(BF16)
    svec = (-CREL * mlp_w * b1 + bias_v @ w1Tm_f).astype(np.float32)
    b2s_eff = float(bias_v @ w2sum + b2sum)

    def bc(vec):
        return np.broadcast_to(
            np.asarray(vec, np.float32)[None, :], (128, D)
        ).copy()

    biasq_d = np.ascontiguousarray(biasq.reshape(2, 128).T)
    biask_d = np.ascontiguousarray(biask.reshape(2, 128).T)

    # ---- b-major bf16 activations ----
    k_bm = np.ascontiguousarray(k.transpose(1, 0, 2).reshape(TOKK, D)).astype(BF16)
    v_bm = np.ascontiguousarray(v.transpose(1, 0, 2).reshape(TOKK, D)).astype(BF16)
    q_bm = q.transpose(1, 0, 2).astype(BF16)             # (B, N, D)

    common = {
        "k_nb": k_bm,
        "v_nb": v_bm,
        "wqc": wqc,
        "wkc": wkc,
        "wvc": wvc,
        "w1Tm": w1Tm,
        "biasq": biasq_d,
        "biask": biask_d,
        "svec_bc": bc(svec),
        "mlb_bc": bc(mlp_b),
        "bv_bc": bc(bias_v),
        "identf": np.eye(128, dtype=np.float32),
        "ones_col": np.ones((128, 1), np.float32),
        "ones_row": np.ones((1, 128), np.float32),
    }
    in_maps = []
    for i in range(NCORES):
        q_sl = np.ascontiguousarray(
            q_bm[:, i * NLOC : (i + 1) * NLOC, :].reshape(TOKQ, D)
        )
        in_maps.append({**common, "q_nb": q_sl})

    return in_maps, {"b2s_eff": b2s_eff}


def assemble(results):
    full = np.zeros((B, N, D), np.float32)
    for i in range(NCORES):
        o = np.asarray(results[i]["out"]).reshape(B, NLOC, D)
        full[:, i * NLOC : (i + 1) * NLOC, :] = o
    return np.ascontiguousarray(full.transpose(1, 0, 2))


def kernel(**inputs):
    from concourse import bass_utils

    in_maps, consts = prepare(**inputs)
    nc = _build(consts)
    nc.compile()
    res = bass_utils.run_bass_kernel_spmd(nc, in_maps, core_ids=list(range(NCORES)))
    global LAST_RESULT
    LAST_RESULT = res
    return assemble([res.results[i] for i in range(NCORES)])
